# revision 7
# baseline (speedup 1.0000x reference)
"""AR video patch transformer forward on 8 Trainium2 NeuronCores.

Strategy: pure data parallelism — each core runs the full 8-layer
transformer on one batch element. Host does patchify/unpatchify and
weight preprocessing (scale folds, padding, lhsT tiling, fp16 cast).

On-chip layout: activations are dim-major (features on partitions,
512 tokens on the free axis), fp32 residual stream with an fp16 mirror
feeding the matmuls. All GEMMs run fp16 x fp16 -> fp32 PSUM.

Attention per head computes S^T = K^T-major scores (k-tokens on
partitions) so softmax-normalized probabilities feed attn@v directly;
masking is handled by block-causal skipping (frames are 64 tokens) plus
one zeroed 64x64 strip per k-tile; denominators ride along as a ones
column appended to V (token-major), and the per-token division happens
once on the 64-row attention output via an exp(-ln(x)) reciprocal and a
K=1 broadcast matmul.
"""

import numpy as np

import concourse.bass as bass
import concourse.mybir as mybir
from concourse import bacc
from concourse.tile import TileContext
from concourse.bass_utils import run_bass_kernel_spmd

F = mybir.ActivationFunctionType
FP16 = mybir.dt.float16
FP32 = mybir.dt.float32

# Model config (hardcoded from the problem spec)
B = 8; T = 8; C = 3; RES = 64; P = 8
D = 1024; NH = 16; HD = 64; NL = 8
INNER = 2730
NP_ = 64           # patches per frame
PD = 192           # patch dim
PDP = 256          # padded patch dim (2 k-tiles)
L = 512            # tokens
EPS = 1e-6
KT = D // 128      # 8
IH = INNER // 2    # 1365 half-inner
IHP = 1408         # padded half-inner (11 tiles)
JT = IHP // 128    # 11
EP9 = 2.0 ** -9    # exact fp16 scalar used for the eps matmul

N_CORES = 8
_CACHE = {}


# ----------------------------------------------------------------------
# host-side preprocessing
# ----------------------------------------------------------------------

def _lhsT_tile(w):
    """[Din, Dout] -> [Dout/128, 128, Din] fp16 lhsT-tiled blocks."""
    din, dout = w.shape
    kt, ot = din // 128, dout // 128
    return np.ascontiguousarray(
        w.reshape(kt, 128, ot, 128).transpose(2, 1, 0, 3).reshape(ot, 128, din)
    ).astype(np.float16)


def _rope_tables(scale):
    """C/S tables [128, L] with the per-dim norm scale folded in
    (scale applied before rotation, matching the reference order)."""
    q = HD // 4  # 16
    inv = 1.0 / (10000.0 ** (np.arange(q, dtype=np.float64) / q))
    t_idx = np.repeat(np.arange(T), NP_)
    s_idx = np.tile(np.arange(NP_), T)
    ang = np.concatenate(
        [t_idx[:, None] * inv[None, :], s_idx[:, None] * inv[None, :]], axis=1
    )  # (L, 32)
    cdm = np.zeros((128, L), np.float64)
    sdm = np.zeros((128, L), np.float64)
    for d in range(128):
        dl = d % 64
        i = dl // 2
        cdm[d] = np.cos(ang[:, i]) * scale[dl]
        sg = -1.0 if d % 2 == 0 else 1.0
        sdm[d] = sg * np.sin(ang[:, i]) * scale[dl ^ 1]
    return cdm.astype(np.float16), sdm.astype(np.float16)


def _prep_weights(inp):
    w = {}
    n1 = inp["norm1_scale"]; n2 = inp["norm2_scale"]
    qk_t = np.empty((NL, 16, 128, D), np.float16)
    wv_p = np.empty((NL, KT, 128, D), np.float16)
    wo_t = np.empty((NL, 8, 128, D), np.float16)
    gu_t = np.empty((NL, 44, 128, D), np.float16)
    dn_t = np.empty((NL, 8, 128, IHP), np.float16)
    for l in range(NL):
        w1 = inp["qkv_w"][l] * n1[l][:, None]
        qk_t[l] = _lhsT_tile(w1[:, :2048])
        wv_p[l] = w1[:, 2048:].reshape(KT, 128, D).astype(np.float16)
        wo_t[l] = _lhsT_tile(inp["out_w"][l])
        g = inp["gate_w"][l] * n2[l][:, None]
        u = inp["up_w"][l] * n2[l][:, None]
        gp = np.zeros((D, 2 * IHP), np.float32)
        up = np.zeros((D, 2 * IHP), np.float32)
        gp[:, :IH] = g[:, :IH]; gp[:, IHP:IHP + IH] = g[:, IH:]
        up[:, :IH] = u[:, :IH]; up[:, IHP:IHP + IH] = u[:, IH:]
        gt = _lhsT_tile(gp); ut = _lhsT_tile(up)
        order = []
        for j in range(JT):
            order += [gt[j], gt[JT + j], ut[j], ut[JT + j]]
        gu_t[l] = np.stack(order)
        dp = np.zeros((IHP, D), np.float32)
        dp[:IH] = inp["down_w"][l]
        dn_t[l] = _lhsT_tile(dp)
    w["qk_t"] = qk_t; w["wv_p"] = wv_p; w["wo_t"] = wo_t
    w["gu_t"] = gu_t; w["dn_t"] = dn_t

    pe = np.zeros((PDP, D), np.float32)
    pe[:PD] = inp["patch_embed_w"]
    w["pe_t"] = _lhsT_tile(pe)
    hw = np.zeros((D, PDP), np.float32)
    hw[:, :PD] = inp["head_w"] * inp["normf_scale"][:, None]
    w["hd_t"] = _lhsT_tile(hw)

    # per-head broadcast matrix (block ones)
    e1 = np.zeros((8, 16, 128), np.float16)
    for t in range(8):
        for dl in range(128):
            e1[t, 2 * t + dl // 64, dl] = 1.0
    w["e1"] = np.ascontiguousarray(e1.transpose(1, 0, 2).reshape(16, 8 * 128))

    w["esc"] = inp["embed_norm_scale"].reshape(1, 8 * 128).astype(np.float16)

    cq = np.empty((NL, 128, L), np.float16); sq = np.empty((NL, 128, L), np.float16)
    ck = np.empty((NL, 128, L), np.float16); sk = np.empty((NL, 128, L), np.float16)
    for l in range(NL):
        cq[l], sq[l] = _rope_tables(inp["q_norm_scale"][l])
        ck[l], sk[l] = _rope_tables(inp["k_norm_scale"][l])
    w["cq"] = cq; w["sq"] = sq; w["ck"] = ck; w["sk"] = sk
    psw = np.zeros((128, 128), np.float16)
    for i in range(128):
        psw[i ^ 1, i] = 1.0
    w["psw"] = psw
    b16 = np.zeros((128, 8 * 16), np.float16)
    for t in range(8):
        for dl in range(128):
            b16[dl, t * 16 + 2 * t + dl // 64] = 1.0 / 64.0
    w["b16"] = b16
    return w


def _patchify(frames_b):
    # (T, C, RES, RES) -> (L, PD)
    h = RES // P
    x = frames_b.reshape(T, C, h, P, h, P)
    x = x.transpose(0, 2, 4, 1, 3, 5).reshape(T * h * h, C * P * P)
    return x


def _unpatchify(tokens):
    # (L, PD) -> (T, C, RES, RES)
    h = RES // P
    y = tokens.reshape(T, h, h, C, P, P)
    return y.transpose(0, 3, 1, 4, 2, 5).reshape(T, C, RES, RES)


# ----------------------------------------------------------------------
# device kernel
# ----------------------------------------------------------------------

def _build(nl=NL):
    nc = bacc.Bacc()
    d = {}
    d["x0T"] = nc.dram_tensor("x0T", [PDP, L], FP16, kind="ExternalInput")
    d["qk_t"] = nc.dram_tensor("qk_t", [NL, 16, 128, D], FP16, kind="ExternalInput")
    d["wv_p"] = nc.dram_tensor("wv_p", [NL, KT, 128, D], FP16, kind="ExternalInput")
    d["wo_t"] = nc.dram_tensor("wo_t", [NL, 8, 128, D], FP16, kind="ExternalInput")
    d["gu_t"] = nc.dram_tensor("gu_t", [NL, 44, 128, D], FP16, kind="ExternalInput")
    d["dn_t"] = nc.dram_tensor("dn_t", [NL, 8, 128, IHP], FP16, kind="ExternalInput")
    d["pe_t"] = nc.dram_tensor("pe_t", [8, 128, PDP], FP16, kind="ExternalInput")
    d["hd_t"] = nc.dram_tensor("hd_t", [2, 128, D], FP16, kind="ExternalInput")
    d["e1"] = nc.dram_tensor("e1", [16, 8 * 128], FP16, kind="ExternalInput")
    d["esc"] = nc.dram_tensor("esc", [1, 8 * 128], FP16, kind="ExternalInput")
    d["cq"] = nc.dram_tensor("cq", [NL, 128, L], FP16, kind="ExternalInput")
    d["sq"] = nc.dram_tensor("sq", [NL, 128, L], FP16, kind="ExternalInput")
    d["ck"] = nc.dram_tensor("ck", [NL, 128, L], FP16, kind="ExternalInput")
    d["sk"] = nc.dram_tensor("sk", [NL, 128, L], FP16, kind="ExternalInput")
    d["psw"] = nc.dram_tensor("psw", [128, 128], FP16, kind="ExternalInput")
    d["b16"] = nc.dram_tensor("b16", [128, 8 * 16], FP16, kind="ExternalInput")
    out_d = nc.dram_tensor("out", [PD, L], FP32, kind="ExternalOutput")

    with TileContext(nc) as tc:
        _emit(nc, tc, d, out_d, nl)
    nc.compile()
    return nc


def _emit(nc, tc, d, out_d, nl):
    import contextlib
    ctx = contextlib.ExitStack()
    with ctx:
        cpool = ctx.enter_context(tc.tile_pool(name="consts", bufs=1))
        xpool = ctx.enter_context(tc.tile_pool(name="x", bufs=1))
        wpool = ctx.enter_context(tc.tile_pool(name="w", bufs=3))
        apool = ctx.enter_context(tc.tile_pool(name="act", bufs=1))
        spool = ctx.enter_context(tc.tile_pool(name="small", bufs=2))
        ps_g = ctx.enter_context(tc.tile_pool(name="psg", bufs=4, space="PSUM"))
        ps_acc = ctx.enter_context(tc.tile_pool(name="psacc", bufs=2, space="PSUM"))
        ps_bc = ctx.enter_context(tc.tile_pool(name="psbc", bufs=2, space="PSUM"))

        # ---- persistent constants ----
        psw = cpool.tile([128, 128], FP16, name="psw")
        nc.sync.dma_start(psw[:], d["psw"][:])
        b16 = cpool.tile([128, 128], FP16, name="b16")
        nc.sync.dma_start(b16[:], d["b16"][:])
        e1s = cpool.tile([16, 8 * 128], FP16, name="e1s")
        nc.sync.dma_start(e1s[:], d["e1"][:])
        escs = cpool.tile([1, 8 * 128], FP16, name="escs")
        nc.sync.dma_start(escs[:], d["esc"][:])
        ones1 = cpool.tile([1, 128], FP16, name="ones1")
        nc.gpsimd.memset(ones1[:], 1.0)
        onesd = cpool.tile([128, 1], FP16, name="onesd")
        nc.gpsimd.memset(onesd[:], 1.0 / 1024.0)
        ep9 = cpool.tile([1, 16], FP16, name="ep9")
        nc.gpsimd.memset(ep9[:], EP9)
        epsb = cpool.tile([128, 1], FP32, name="epsb")
        nc.gpsimd.memset(epsb[:], EPS)
        identf = cpool.tile([1, 1], FP32, name="identf")
        nc.gpsimd.memset(identf[:], 1.0)

        # residual stream
        xs = [xpool.tile([128, L], FP32, name=f"x{t}") for t in range(8)]
        x16 = [xpool.tile([128, L], FP16, name=f"m{t}") for t in range(8)]

        def rms_mean(src16, ktiles, name):
            """mean over features of src16 tiles -> psum [1, L]."""
            mean = ps_acc.tile([1, L], FP32, name=f"mean_{name}", tag="acc")
            for t in range(ktiles):
                sq = spool.tile([128, L], FP16, name=f"sq_{name}_{t}", tag="sq",
                                bufs=2)
                nc.scalar.activation(sq[:], src16[t][:], F.Square)
                nc.tensor.matmul(mean[:], onesd[:], sq[:],
                                 start=(t == 0), stop=(t == ktiles - 1))
            return mean

        def bcast_mul_into(dst, srcs, lhs_vecs, rhs, cast16=None, name=""):
            """dst[t] = srcs[t] * (lhs_vecs[t].T @ rhs) for 8 tiles."""
            for t in range(8):
                bc = ps_bc.tile([128, L], FP32, name=f"bc_{name}_{t}", tag="bc")
                nc.tensor.matmul(bc[:], lhs_vecs(t), rhs[:], start=True,
                                 stop=True)
                nc.vector.tensor_mul(dst[t][:], srcs[t][:], bc[:])
                if cast16 is not None:
                    nc.vector.tensor_copy(cast16[t][:], dst[t][:])

        # ---------- patch embed ----------
        x0 = apool.tile([128, 2, L], FP16, name="x0")
        nc.sync.dma_start(x0[:], d["x0T"].rearrange("(k p) t -> p k t", p=128))
        for t in range(8):
            wt = wpool.tile([128, PDP], FP16, name=f"pe_w{t}", tag="w")
            nc.sync.dma_start(wt[:], d["pe_t"][t])
            ps = ps_g.tile([128, L], FP32, name=f"pe_ps{t}", tag="g")
            for k in range(2):
                nc.tensor.matmul(ps[:], wt[:, k * 128:(k + 1) * 128],
                                 x0[:, k, :], start=(k == 0), stop=(k == 1))
            nc.scalar.activation(xs[t][:], ps[:], F.Copy)
            nc.vector.tensor_copy(x16[t][:], xs[t][:])
        mean_e = rms_mean(x16, 8, "emb")
        re_sb = spool.tile([1, L], FP16, name="re_sb", tag="r16")
        nc.scalar.activation(re_sb[:], mean_e[:], F.Abs_reciprocal_sqrt,
                             bias=epsb[0:1])
        bcast_mul_into(xs, xs, lambda t: escs[0:1, t * 128:(t + 1) * 128], re_sb,
                       cast16=x16, name="emb")

        # ---------- layers ----------
        for l in range(nl):
            _layer(nc, tc, d, l, xs, x16, cpool, wpool, apool, spool,
                   ps_g, ps_acc, ps_bc,
                   psw, b16, e1s, ones1, onesd, ep9, epsb,
                   identf, rms_mean)

        # ---------- final norm + head ----------
        mean_f = rms_mean(x16, 8, "fin")
        rf_sb = spool.tile([1, L], FP16, name="rf_sb", tag="r16")
        nc.scalar.activation(rf_sb[:], mean_f[:], F.Abs_reciprocal_sqrt,
                             bias=epsb[0:1])
        hN = [apool.tile([128, L], FP16, name=f"hN{t}", tag=f"h2{t}")
              for t in range(8)]
        for t in range(8):
            bc = ps_bc.tile([128, L], FP32, name=f"bc_fin_{t}", tag="bc")
            nc.tensor.matmul(bc[:], ones1[:], rf_sb[:], start=True, stop=True)
            nc.vector.tensor_mul(hN[t][:], x16[t][:], bc[:])
        for o in range(2):
            wt = wpool.tile([128, D], FP16, name=f"hd_w{o}", tag="w")
            nc.sync.dma_start(wt[:], d["hd_t"][o])
            ps = ps_g.tile([128, L], FP32, name=f"hd_ps{o}", tag="g")
            for k in range(KT):
                nc.tensor.matmul(ps[:], wt[:, k * 128:(k + 1) * 128],
                                 hN[k][:], start=(k == 0), stop=(k == KT - 1))
            rows = 128 if o == 0 else PD - 128
            ot = apool.tile([128, L], FP32, name=f"hd_o{o}")
            nc.scalar.activation(ot[:rows, :], ps[:rows, :], F.Copy)
            nc.sync.dma_start(out_d[o * 128:o * 128 + rows, :], ot[:rows, :])


def _layer(nc, tc, d, l, xs, x16, cpool, wpool, apool, spool,
           ps_g, ps_acc, ps_bc,
           psw, b16, e1s, ones1, onesd, ep9, epsb,
           identf, rms_mean):
    # per-layer rope/scale tables
    rtab = spool.tile([128, 4, L], FP16, name=f"rtab_{l}", tag="rtab", bufs=1)
    nc.sync.dma_start(rtab[:, 0, :], d["cq"][l])
    nc.sync.dma_start(rtab[:, 1, :], d["sq"][l])
    nc.sync.dma_start(rtab[:, 2, :], d["ck"][l])
    nc.sync.dma_start(rtab[:, 3, :], d["sk"][l])
    # ---- RMS1 statistics ----
    mean1 = rms_mean(x16, 8, f"l{l}a")
    zs = spool.tile([1, L], FP16, name=f"zs_{l}", tag="r16")
    nc.scalar.activation(zs[:], mean1[:], F.Identity, scale=EPS / EP9)
    r1 = spool.tile([1, L], FP32, name=f"r1_{l}", tag="r32")
    nc.scalar.activation(r1[:], mean1[:], F.Abs_reciprocal_sqrt, bias=epsb[0:1])
    # transpose r1 -> rT [128, 4]
    trp = ps_bc.tile([128, 4], FP32, name=f"trp_{l}", tag="bc")
    for b in range(4):
        nc.tensor.transpose(trp[:, b:b + 1], r1[:, b * 128:(b + 1) * 128],
                            identf[:])
    rT = spool.tile([128, 4], FP32, name=f"rT_{l}", tag="rT")
    nc.vector.tensor_copy(rT[:], trp[:])

    # ---- qkv GEMMs ----
    qraw = []
    msq_q = ps_acc.tile([16, L], FP32, name=f"msqq_{l}", tag="acc")
    msq_k = ps_acc.tile([16, L], FP32, name=f"msqk_{l}", tag="acc")
    for grp in range(4):
        wt = wpool.tile([128, 4, D], FP16, name=f"qkw_{l}_{grp}", tag="w")
        nc.sync.dma_start(wt[:], d["qk_t"][l, grp * 4:(grp + 1) * 4]
                          .rearrange("g p n -> p g n"))
        for gi in range(4):
            ot = grp * 4 + gi
            ps = ps_g.tile([128, L], FP32, name=f"qk_ps_{l}_{ot}", tag="g")
            for k in range(KT):
                nc.tensor.matmul(ps[:], wt[:, gi, k * 128:(k + 1) * 128],
                                 x16[k][:], start=(k == 0), stop=(k == KT - 1))
            qr = spool.tile([128, L], FP16, name=f"qraw_{l}_{ot}", tag="qraw",
                            bufs=16)
            nc.scalar.activation(qr[:], ps[:], F.Copy)
            qraw.append(qr)
            sq = spool.tile([128, L], FP16, name=f"qsq_{l}_{ot}", tag="sq",
                            bufs=2)
            nc.scalar.activation(sq[:], ps[:], F.Square)
            tt = ot % 8
            msq = msq_q if ot < 8 else msq_k
            nc.tensor.matmul(msq[:], b16[:, tt * 16:(tt + 1) * 16], sq[:],
                             start=(tt == 0), stop=False)
    # eps correction: msq += ep9 * zs  (= eps * ir2)
    nc.tensor.matmul(msq_q[:], ep9[:], zs[:], start=False, stop=True)
    nc.tensor.matmul(msq_k[:], ep9[:], zs[:], start=False, stop=True)

    # v (token-major) with r1 scaling
    vsb = [apool.tile([128, 16 * 65], FP16, name=f"vsb_{l}_{b}", tag=f"vsb{b}")
           for b in range(4)]
    wva = wpool.tile([128, 4, D], FP16, name=f"vwa_{l}", tag="w")
    nc.sync.dma_start(wva[:], d["wv_p"][l, 0:4].rearrange("k p n -> p k n"))
    wvb = wpool.tile([128, 4, D], FP16, name=f"vwb_{l}", tag="w")
    nc.sync.dma_start(wvb[:], d["wv_p"][l, 4:8].rearrange("k p n -> p k n"))
    for b in range(4):
        nc.gpsimd.memset(
            vsb[b][:].rearrange("p (h c) -> p h c", c=65)[:, :, 64:65], 1.0)
        for n in range(2):
            ps = ps_g.tile([128, 512], FP32, name=f"v_ps_{l}_{b}_{n}", tag="g")
            for k in range(KT):
                wv = wva if k < 4 else wvb
                nc.tensor.matmul(ps[:], x16[k][:, b * 128:(b + 1) * 128],
                                 wv[:, k % 4, n * 512:(n + 1) * 512],
                                 start=(k == 0), stop=(k == KT - 1))
            dst = vsb[b][:].rearrange("p (h c) -> p h c", c=65)[:, n * 8:(n + 1) * 8, 0:64]
            nc.vector.tensor_scalar_mul(dst, ps[:], rT[:, b:b + 1])

    # ---- per-head q/k norm multipliers ----
    aq = spool.tile([16, L], FP16, name=f"aq_{l}", tag="a16")
    tq = spool.tile([16, L], FP32, name=f"tq_{l}", tag="a32")
    nc.scalar.activation(tq[:], msq_q[:], F.Ln)
    nc.scalar.activation(aq[:], tq[:], F.Exp, scale=-0.5)
    ak = spool.tile([16, L], FP16, name=f"ak_{l}", tag="a16")
    tk = spool.tile([16, L], FP32, name=f"tk_{l}", tag="a32")
    nc.scalar.activation(tk[:], msq_k[:], F.Ln)
    nc.scalar.activation(ak[:], tk[:], F.Exp, scale=-0.5)

    # ---- rope + norm apply ----
    qf = []
    for ot in range(16):
        t = ot % 8
        alpha = aq if ot < 8 else ak
        ci, si = (0, 1) if ot < 8 else (2, 3)
        bc = ps_bc.tile([128, L], FP32, name=f"rbc_{l}_{ot}", tag="bc")
        nc.tensor.matmul(bc[:], e1s[:, t * 128:(t + 1) * 128],
                         alpha[:], start=True, stop=True)
        sw = ps_bc.tile([128, L], FP32, name=f"rsw_{l}_{ot}", tag="bc")
        nc.tensor.matmul(sw[:], psw[:], qraw[ot][:], start=True, stop=True)
        u1 = spool.tile([128, L], FP16, name=f"u1_{l}_{ot}", tag="u1")
        nc.vector.tensor_mul(u1[:], qraw[ot][:], rtab[:, ci, :])
        u2 = spool.tile([128, L], FP16, name=f"u2_{l}_{ot}", tag="u2")
        nc.vector.tensor_mul(u2[:], sw[:], rtab[:, si, :])
        nc.vector.tensor_add(u1[:], u1[:], u2[:])
        qt = apool.tile([128, L], FP16, name=f"qf_{l}_{ot}", tag=f"qf{ot}")
        nc.vector.tensor_mul(qt[:], u1[:], bc[:])
        qf.append(qt)

    # ---- attention ----
    rden = spool.tile([1, 16 * L], FP16, name=f"rden_{l}", tag="den", bufs=1)
    oun = [apool.tile([128, L], FP16, name=f"oun_{l}_{t}", tag=f"oun{t}")
           for t in range(8)]
    for h in range(16):
        ti = h // 2
        r0 = (h % 2) * 64
        oe = ps_acc.tile([65, L], FP32, name=f"oe_{l}_{h}", tag="acc")
        for kt in range(4):
            q0 = 128 * kt
            st = ps_g.tile([128, L], FP32, name=f"st_{l}_{h}_{kt}", tag="g")
            nc.tensor.matmul(st[:, q0:], qf[8 + ti][r0:r0 + 64, kt * 128:(kt + 1) * 128],
                             qf[ti][r0:r0 + 64, q0:], start=True, stop=True)
            est = spool.tile([128, L], FP16, name=f"est_{l}_{h}_{kt}",
                             tag="est", bufs=4)
            nc.scalar.activation(est[0:64, q0:], st[0:64, q0:], F.Exp,
                                 scale=0.125)
            if q0 + 64 < L:
                nc.scalar.activation(est[64:128, q0 + 64:], st[64:128, q0 + 64:],
                                     F.Exp, scale=0.125)
            nc.any.memset(est[64:128, q0:q0 + 64], 0.0)
            nc.tensor.matmul(oe[:, q0:], vsb[kt][:, h * 65:(h + 1) * 65],
                             est[:, q0:], start=(kt == 0), stop=(kt == 3))
        lnh = spool.tile([1, L], FP32, name=f"lnh_{l}_{h}", tag="lnh")
        nc.scalar.activation(lnh[:], oe[64:65, :], F.Ln)
        nc.scalar.activation(rden[0:1, h * L:(h + 1) * L], lnh[:], F.Exp,
                             scale=-1.0)
        nc.scalar.activation(oun[ti][r0:r0 + 64, :], oe[0:64, :], F.Copy)

    # normalize + out projection
    of = [spool.tile([128, L], FP16, name=f"of_{l}_{t}", tag=f"of{t}", bufs=1)
          for t in range(8)]
    for t in range(8):
        bc = ps_bc.tile([128, L], FP32, name=f"nbc_{l}_{t}", tag="bc")
        nc.tensor.matmul(bc[0:64, :], ones1[0:1, 0:64],
                         rden[0:1, (2 * t) * L:(2 * t + 1) * L],
                         start=True, stop=True)
        nc.tensor.matmul(bc[64:128, :], ones1[0:1, 0:64],
                         rden[0:1, (2 * t + 1) * L:(2 * t + 2) * L],
                         start=True, stop=True)
        nc.vector.tensor_mul(of[t][:], oun[t][:], bc[:])
    for grp in range(2):
        wt = wpool.tile([128, 4, D], FP16, name=f"wo_{l}_{grp}", tag="w")
        nc.sync.dma_start(wt[:], d["wo_t"][l, grp * 4:(grp + 1) * 4]
                          .rearrange("g p n -> p g n"))
        for gi in range(4):
            t = grp * 4 + gi
            ps = ps_g.tile([128, L], FP32, name=f"xa_ps_{l}_{t}", tag="g")
            for k in range(KT):
                nc.tensor.matmul(ps[:], wt[:, gi, k * 128:(k + 1) * 128],
                                 of[k][:], start=(k == 0), stop=(k == KT - 1))
            nc.vector.tensor_add(xs[t][:], xs[t][:], ps[:])
            nc.vector.tensor_copy(x16[t][:], xs[t][:])

    # ---- MLP ----
    mean2 = rms_mean(x16, 8, f"l{l}b")
    r2 = spool.tile([1, L], FP16, name=f"r2_{l}", tag="r16")
    nc.scalar.activation(r2[:], mean2[:], F.Abs_reciprocal_sqrt, bias=epsb[0:1])
    h2 = [apool.tile([128, L], FP16, name=f"h2_{l}_{t}", tag=f"h2{t}")
          for t in range(8)]
    bch = ps_bc.tile([128, L], FP32, name=f"bch_{l}", tag="bc")
    nc.tensor.matmul(bch[:], ones1[:], r2[:], start=True, stop=True)
    for t in range(8):
        nc.vector.tensor_mul(h2[t][:], x16[t][:], bch[:])

    pj = []
    for j in range(JT):
        wt = wpool.tile([128, 4, D], FP16, name=f"gu_{l}_{j}", tag="w")
        nc.sync.dma_start(wt[:], d["gu_t"][l, j * 4:(j + 1) * 4]
                          .rearrange("g p n -> p g n"))
        pss = []
        for gi in range(4):
            ps = ps_g.tile([128, L], FP32, name=f"gu_ps_{l}_{j}_{gi}", tag="g")
            for k in range(KT):
                nc.tensor.matmul(ps[:], wt[:, gi, k * 128:(k + 1) * 128],
                                 h2[k][:], start=(k == 0), stop=(k == KT - 1))
            pss.append(ps)
        sg1 = spool.tile([128, L], FP16, name=f"sg1_{l}_{j}", tag="sg1")
        nc.scalar.activation(sg1[:], pss[0][:], F.Silu)
        sg2 = spool.tile([128, L], FP16, name=f"sg2_{l}_{j}", tag="sg2")
        nc.scalar.activation(sg2[:], pss[1][:], F.Silu)
        ta = spool.tile([128, L], FP16, name=f"ta_{l}_{j}", tag="ta")
        nc.vector.tensor_mul(ta[:], sg1[:], pss[2][:])
        tb = spool.tile([128, L], FP16, name=f"tb_{l}_{j}", tag="tb")
        nc.vector.tensor_mul(tb[:], sg2[:], pss[3][:])
        p = spool.tile([128, L], FP16, name=f"p_{l}_{j}", tag=f"p{j}", bufs=1)
        nc.vector.tensor_add(p[:], ta[:], tb[:])
        pj.append(p)

    for grp in range(4):
        wt = wpool.tile([128, 2, IHP], FP16, name=f"dn_{l}_{grp}", tag="w")
        nc.sync.dma_start(wt[:], d["dn_t"][l, grp * 2:(grp + 1) * 2]
                          .rearrange("g p n -> p g n"))
        for gi in range(2):
            t = grp * 2 + gi
            ps = ps_g.tile([128, L], FP32, name=f"dn_ps_{l}_{t}", tag="g")
            for j in range(JT):
                nc.tensor.matmul(ps[:], wt[:, gi, j * 128:(j + 1) * 128],
                                 pj[j][:], start=(j == 0), stop=(j == JT - 1))
            nc.vector.tensor_add(xs[t][:], xs[t][:], ps[:])
            nc.vector.tensor_copy(x16[t][:], xs[t][:])


# ----------------------------------------------------------------------
# entry point
# ----------------------------------------------------------------------

def _get_nc(nl=NL):
    if nl not in _CACHE:
        _CACHE[nl] = _build(nl)
    return _CACHE[nl]


def run(inputs, nl=NL, trace=False):
    inputs = {k: np.asarray(v) for k, v in inputs.items()}
    w = _prep_weights(inputs)
    in_maps = []
    for b in range(N_CORES):
        tok = _patchify(inputs["frames"][b]).astype(np.float32)
        x0T = np.zeros((PDP, L), np.float16)
        x0T[:PD] = tok.T.astype(np.float16)
        m = {"x0T": x0T, "qk_t": w["qk_t"], "wv_p": w["wv_p"],
             "wo_t": w["wo_t"], "gu_t": w["gu_t"], "dn_t": w["dn_t"],
             "pe_t": w["pe_t"], "hd_t": w["hd_t"],
             "e1": w["e1"], "esc": w["esc"],
             "cq": w["cq"], "sq": w["sq"], "ck": w["ck"], "sk": w["sk"],
             "psw": w["psw"], "b16": w["b16"]}
        in_maps.append(m)
    nc = _get_nc(nl)
    res = run_bass_kernel_spmd(nc, in_maps, list(range(N_CORES)), trace=trace)
    outs = []
    for b in range(N_CORES):
        tok = res.results[b]["out"].T  # (L, PD)
        outs.append(_unpatchify(tok))
    return np.stack(outs).astype(np.float32), res


def kernel(**inputs) -> np.ndarray:
    out, _ = run(inputs)
    return out


# revision 12
# speedup vs baseline: 1.2856x; 1.2856x over previous
"""AR video patch transformer forward on 8 Trainium2 NeuronCores.

Strategy: pure data parallelism — each core runs the full 8-layer
transformer on one batch element. Host does patchify/unpatchify and
weight preprocessing (scale folds, padding, lhsT tiling, fp16 cast).

On-chip layout: activations are dim-major (features on partitions,
512 tokens on the free axis), fp32 residual stream with an fp16 mirror
feeding the matmuls. All GEMMs run fp16 x fp16 -> fp32 PSUM.

Attention per head computes S^T = K^T-major scores (k-tokens on
partitions) so softmax-normalized probabilities feed attn@v directly;
masking is handled by block-causal skipping (frames are 64 tokens) plus
one zeroed 64x64 strip per k-tile; denominators ride along as a ones
column appended to V (token-major), and the per-token division happens
once on the 64-row attention output via an exp(-ln(x)) reciprocal and a
K=1 broadcast matmul.
"""

import numpy as np

import concourse.bass as bass
import concourse.mybir as mybir
from concourse import bacc
from concourse.tile import TileContext
from concourse.bass_utils import run_bass_kernel_spmd

F = mybir.ActivationFunctionType
FP16 = mybir.dt.float16
FP32 = mybir.dt.float32

# Model config (hardcoded from the problem spec)
B = 8; T = 8; C = 3; RES = 64; P = 8
D = 1024; NH = 16; HD = 64; NL = 8
INNER = 2730
NP_ = 64           # patches per frame
PD = 192           # patch dim
PDP = 256          # padded patch dim (2 k-tiles)
L = 512            # tokens
EPS = 1e-6
KT = D // 128      # 8
IH = INNER // 2    # 1365 half-inner
IHP = 1408         # padded half-inner (11 tiles)
JT = IHP // 128    # 11
EP9 = 2.0 ** -9    # exact fp16 scalar used for the eps matmul

N_CORES = 8
_CACHE = {}


# ----------------------------------------------------------------------
# host-side preprocessing
# ----------------------------------------------------------------------

def _lhsT_tile(w):
    """[Din, Dout] -> [Dout/128, 128, Din] fp16 lhsT-tiled blocks."""
    din, dout = w.shape
    kt, ot = din // 128, dout // 128
    return np.ascontiguousarray(
        w.reshape(kt, 128, ot, 128).transpose(2, 1, 0, 3).reshape(ot, 128, din)
    ).astype(np.float16)


def _rope_tables(scale):
    """C/S tables [128, L] with the per-dim norm scale folded in
    (scale applied before rotation, matching the reference order)."""
    q = HD // 4  # 16
    inv = 1.0 / (10000.0 ** (np.arange(q, dtype=np.float64) / q))
    t_idx = np.repeat(np.arange(T), NP_)
    s_idx = np.tile(np.arange(NP_), T)
    ang = np.concatenate(
        [t_idx[:, None] * inv[None, :], s_idx[:, None] * inv[None, :]], axis=1
    )  # (L, 32)
    cdm = np.zeros((128, L), np.float64)
    sdm = np.zeros((128, L), np.float64)
    for d in range(128):
        dl = d % 64
        i = dl // 2
        cdm[d] = np.cos(ang[:, i]) * scale[dl]
        sg = -1.0 if d % 2 == 0 else 1.0
        sdm[d] = sg * np.sin(ang[:, i]) * scale[dl ^ 1]
    return cdm.astype(np.float16), sdm.astype(np.float16)


def _prep_weights(inp):
    w = {}
    n1 = inp["norm1_scale"]; n2 = inp["norm2_scale"]
    qk_t = np.empty((NL, 16, 128, D), np.float16)
    wv_p = np.empty((NL, KT, 128, D), np.float16)
    wo_t = np.empty((NL, 8, 128, D), np.float16)
    gu_t = np.empty((NL, 44, 128, D), np.float16)
    dn_t = np.empty((NL, 8, 128, IHP), np.float16)
    for l in range(NL):
        w1 = inp["qkv_w"][l] * n1[l][:, None]
        qk_t[l] = _lhsT_tile(w1[:, :2048])
        wv_p[l] = w1[:, 2048:].reshape(KT, 128, D).astype(np.float16)
        wo_t[l] = _lhsT_tile(inp["out_w"][l])
        g = inp["gate_w"][l] * n2[l][:, None]
        u = inp["up_w"][l] * n2[l][:, None]
        gp = np.zeros((D, 2 * IHP), np.float32)
        up = np.zeros((D, 2 * IHP), np.float32)
        gp[:, :IH] = g[:, :IH]; gp[:, IHP:IHP + IH] = g[:, IH:]
        up[:, :IH] = u[:, :IH]; up[:, IHP:IHP + IH] = u[:, IH:]
        gt = _lhsT_tile(gp); ut = _lhsT_tile(up)
        order = []
        for j in range(JT):
            order += [gt[j], gt[JT + j], ut[j], ut[JT + j]]
        gu_t[l] = np.stack(order)
        dp = np.zeros((IHP, D), np.float32)
        dp[:IH] = inp["down_w"][l]
        dn_t[l] = _lhsT_tile(dp)
    w["qk_t"] = qk_t; w["wv_p"] = wv_p; w["wo_t"] = wo_t
    w["gu_t"] = gu_t; w["dn_t"] = dn_t

    pe = np.zeros((PDP, D), np.float32)
    pe[:PD] = inp["patch_embed_w"]
    w["pe_t"] = _lhsT_tile(pe)
    hw = np.zeros((D, PDP), np.float32)
    hw[:, :PD] = inp["head_w"] * inp["normf_scale"][:, None]
    w["hd_t"] = _lhsT_tile(hw)

    # per-head broadcast matrix (block ones)
    e1 = np.zeros((8, 16, 128), np.float16)
    for t in range(8):
        for dl in range(128):
            e1[t, 2 * t + dl // 64, dl] = 1.0
    w["e1"] = np.ascontiguousarray(e1.transpose(1, 0, 2).reshape(16, 8 * 128))

    w["esc"] = inp["embed_norm_scale"].reshape(1, 8 * 128).astype(np.float16)

    cq = np.empty((NL, 128, L), np.float16); sq = np.empty((NL, 128, L), np.float16)
    ck = np.empty((NL, 128, L), np.float16); sk = np.empty((NL, 128, L), np.float16)
    for l in range(NL):
        cq[l], sq[l] = _rope_tables(inp["q_norm_scale"][l])
        ck[l], sk[l] = _rope_tables(inp["k_norm_scale"][l])
    w["cq"] = cq; w["sq"] = sq; w["ck"] = ck; w["sk"] = sk
    psw = np.zeros((128, 128), np.float16)
    for i in range(128):
        psw[i ^ 1, i] = 1.0
    w["psw"] = psw
    b16 = np.zeros((128, 8 * 16), np.float16)
    for t in range(8):
        for dl in range(128):
            b16[dl, t * 16 + 2 * t + dl // 64] = 1.0 / 64.0
    w["b16"] = b16
    return w


def _patchify(frames_b):
    # (T, C, RES, RES) -> (L, PD)
    h = RES // P
    x = frames_b.reshape(T, C, h, P, h, P)
    x = x.transpose(0, 2, 4, 1, 3, 5).reshape(T * h * h, C * P * P)
    return x


def _unpatchify(tokens):
    # (L, PD) -> (T, C, RES, RES)
    h = RES // P
    y = tokens.reshape(T, h, h, C, P, P)
    return y.transpose(0, 3, 1, 4, 2, 5).reshape(T, C, RES, RES)


# ----------------------------------------------------------------------
# device kernel
# ----------------------------------------------------------------------

def _build(nl=NL):
    nc = bacc.Bacc()
    d = {}
    d["x0T"] = nc.dram_tensor("x0T", [PDP, L], FP16, kind="ExternalInput")
    d["qk_t"] = nc.dram_tensor("qk_t", [NL, 16, 128, D], FP16, kind="ExternalInput")
    d["wv_p"] = nc.dram_tensor("wv_p", [NL, KT, 128, D], FP16, kind="ExternalInput")
    d["wo_t"] = nc.dram_tensor("wo_t", [NL, 8, 128, D], FP16, kind="ExternalInput")
    d["gu_t"] = nc.dram_tensor("gu_t", [NL, 44, 128, D], FP16, kind="ExternalInput")
    d["dn_t"] = nc.dram_tensor("dn_t", [NL, 8, 128, IHP], FP16, kind="ExternalInput")
    d["pe_t"] = nc.dram_tensor("pe_t", [8, 128, PDP], FP16, kind="ExternalInput")
    d["hd_t"] = nc.dram_tensor("hd_t", [2, 128, D], FP16, kind="ExternalInput")
    d["e1"] = nc.dram_tensor("e1", [16, 8 * 128], FP16, kind="ExternalInput")
    d["esc"] = nc.dram_tensor("esc", [1, 8 * 128], FP16, kind="ExternalInput")
    d["cq"] = nc.dram_tensor("cq", [NL, 128, L], FP16, kind="ExternalInput")
    d["sq"] = nc.dram_tensor("sq", [NL, 128, L], FP16, kind="ExternalInput")
    d["ck"] = nc.dram_tensor("ck", [NL, 128, L], FP16, kind="ExternalInput")
    d["sk"] = nc.dram_tensor("sk", [NL, 128, L], FP16, kind="ExternalInput")
    d["psw"] = nc.dram_tensor("psw", [128, 128], FP16, kind="ExternalInput")
    d["b16"] = nc.dram_tensor("b16", [128, 8 * 16], FP16, kind="ExternalInput")
    out_d = nc.dram_tensor("out", [PD, L], FP32, kind="ExternalOutput")

    with TileContext(nc) as tc:
        _emit(nc, tc, d, out_d, nl)
    nc.compile()
    return nc


def _emit(nc, tc, d, out_d, nl):
    import contextlib
    ctx = contextlib.ExitStack()
    with ctx:
        cpool = ctx.enter_context(tc.tile_pool(name="consts", bufs=1))
        xpool = ctx.enter_context(tc.tile_pool(name="x", bufs=1))
        wpool = ctx.enter_context(tc.tile_pool(name="w", bufs=3))
        apool = ctx.enter_context(tc.tile_pool(name="act", bufs=1))
        spool = ctx.enter_context(tc.tile_pool(name="small", bufs=2))
        ps_g = ctx.enter_context(tc.tile_pool(name="psg", bufs=4, space="PSUM"))
        ps_acc = ctx.enter_context(tc.tile_pool(name="psacc", bufs=2, space="PSUM"))
        ps_bc = ctx.enter_context(tc.tile_pool(name="psbc", bufs=2, space="PSUM"))

        # ---- persistent constants ----
        psw = cpool.tile([128, 128], FP16, name="psw")
        nc.sync.dma_start(psw[:], d["psw"][:])
        b16 = cpool.tile([128, 128], FP16, name="b16")
        nc.sync.dma_start(b16[:], d["b16"][:])
        e1s = cpool.tile([16, 8 * 128], FP16, name="e1s")
        nc.sync.dma_start(e1s[:], d["e1"][:])
        escs = cpool.tile([1, 8 * 128], FP16, name="escs")
        nc.sync.dma_start(escs[:], d["esc"][:])
        ones1 = cpool.tile([1, 128], FP16, name="ones1")
        nc.gpsimd.memset(ones1[:], 1.0)
        o16c = cpool.tile([97, 64], FP16, name="o16c")
        nc.gpsimd.memset(o16c[:], 1.0 / 16.0)
        onesd = cpool.tile([128, 1], FP16, name="onesd")
        nc.gpsimd.memset(onesd[:], 1.0 / 1024.0)
        ep9 = cpool.tile([1, 16], FP16, name="ep9")
        nc.gpsimd.memset(ep9[:], EP9)
        epsb = cpool.tile([128, 1], FP32, name="epsb")
        nc.gpsimd.memset(epsb[:], EPS)
        identf = cpool.tile([1, 1], FP32, name="identf")
        nc.gpsimd.memset(identf[:], 1.0)

        # residual stream
        xs = [xpool.tile([128, L], FP32, name=f"x{t}") for t in range(8)]
        x16 = [xpool.tile([128, L], FP16, name=f"m{t}") for t in range(8)]

        def rms_mean(src16, ktiles, name):
            """mean over features of src16 tiles -> psum [1, L]."""
            mean = ps_acc.tile([1, L], FP32, name=f"mean_{name}", tag="acc")
            for t in range(ktiles):
                sq = spool.tile([128, L], FP16, name=f"sq_{name}_{t}", tag="sq",
                                bufs=2)
                nc.scalar.activation(sq[:], src16[t][:], F.Square)
                nc.tensor.matmul(mean[:], onesd[:], sq[:],
                                 start=(t == 0), stop=(t == ktiles - 1))
            return mean

        def bcast_mul_into(dst, srcs, lhs_vecs, rhs, cast16=None, name=""):
            """dst[t] = srcs[t] * (lhs_vecs[t].T @ rhs) for 8 tiles."""
            for t in range(8):
                bc = ps_bc.tile([128, L], FP32, name=f"bc_{name}_{t}", tag="bc")
                nc.tensor.matmul(bc[:], lhs_vecs(t), rhs[:], start=True,
                                 stop=True)
                nc.vector.tensor_mul(dst[t][:], srcs[t][:], bc[:])
                if cast16 is not None:
                    nc.vector.tensor_copy(cast16[t][:], dst[t][:])

        # ---------- patch embed ----------
        x0 = apool.tile([128, 2, L], FP16, name="x0")
        nc.sync.dma_start(x0[:], d["x0T"].rearrange("(k p) t -> p k t", p=128))
        for t in range(8):
            wt = wpool.tile([128, PDP], FP16, name=f"pe_w{t}", tag="w")
            nc.sync.dma_start(wt[:], d["pe_t"][t])
            ps = ps_g.tile([128, L], FP32, name=f"pe_ps{t}", tag="g")
            for k in range(2):
                nc.tensor.matmul(ps[:], wt[:, k * 128:(k + 1) * 128],
                                 x0[:, k, :], start=(k == 0), stop=(k == 1))
            nc.scalar.activation(xs[t][:], ps[:], F.Copy)
            nc.vector.tensor_copy(x16[t][:], xs[t][:])
        mean_e = rms_mean(x16, 8, "emb")
        re_sb = spool.tile([1, L], FP16, name="re_sb", tag="r16")
        nc.scalar.activation(re_sb[:], mean_e[:], F.Abs_reciprocal_sqrt,
                             bias=epsb[0:1])
        bcast_mul_into(xs, xs, lambda t: escs[0:1, t * 128:(t + 1) * 128], re_sb,
                       cast16=x16, name="emb")

        # ---------- layers ----------
        for l in range(nl):
            _layer(nc, tc, d, l, xs, x16, cpool, wpool, apool, spool,
                   ps_g, ps_acc, ps_bc,
                   psw, b16, e1s, ones1, o16c, onesd, ep9, epsb,
                   identf, rms_mean)

        # ---------- final norm + head ----------
        mean_f = rms_mean(x16, 8, "fin")
        rf_sb = spool.tile([1, L], FP16, name="rf_sb", tag="r16")
        nc.scalar.activation(rf_sb[:], mean_f[:], F.Abs_reciprocal_sqrt,
                             bias=epsb[0:1])
        hN = [apool.tile([128, L], FP16, name=f"hN{t}", tag=f"h2{t}")
              for t in range(8)]
        for t in range(8):
            bc = ps_bc.tile([128, L], FP32, name=f"bc_fin_{t}", tag="bc")
            nc.tensor.matmul(bc[:], ones1[:], rf_sb[:], start=True, stop=True)
            nc.vector.tensor_mul(hN[t][:], x16[t][:], bc[:])
        for o in range(2):
            wt = wpool.tile([128, D], FP16, name=f"hd_w{o}", tag="w")
            nc.sync.dma_start(wt[:], d["hd_t"][o])
            ps = ps_g.tile([128, L], FP32, name=f"hd_ps{o}", tag="g")
            for k in range(KT):
                nc.tensor.matmul(ps[:], wt[:, k * 128:(k + 1) * 128],
                                 hN[k][:], start=(k == 0), stop=(k == KT - 1))
            rows = 128 if o == 0 else PD - 128
            ot = apool.tile([128, L], FP32, name=f"hd_o{o}")
            nc.scalar.activation(ot[:rows, :], ps[:rows, :], F.Copy)
            nc.sync.dma_start(out_d[o * 128:o * 128 + rows, :], ot[:rows, :])


def _layer(nc, tc, d, l, xs, x16, cpool, wpool, apool, spool,
           ps_g, ps_acc, ps_bc,
           psw, b16, e1s, ones1, o16c, onesd, ep9, epsb,
           identf, rms_mean):
    # per-layer rope/scale tables
    rtab = spool.tile([128, 4, L], FP16, name=f"rtab_{l}", tag="rtab", bufs=1)
    nc.sync.dma_start(rtab[:, 0, :], d["cq"][l])
    nc.sync.dma_start(rtab[:, 1, :], d["sq"][l])
    nc.sync.dma_start(rtab[:, 2, :], d["ck"][l])
    nc.sync.dma_start(rtab[:, 3, :], d["sk"][l])
    # ---- RMS1 statistics ----
    mean1 = rms_mean(x16, 8, f"l{l}a")
    zs = spool.tile([1, L], FP16, name=f"zs_{l}", tag="r16")
    nc.vector.tensor_scalar_mul(zs[:], mean1[:], EPS / EP9)
    r1 = spool.tile([1, L], FP32, name=f"r1_{l}", tag="r32")
    nc.scalar.activation(r1[:], mean1[:], F.Abs_reciprocal_sqrt, bias=epsb[0:1])
    # transpose r1 -> rT [128, 4]
    trp = ps_bc.tile([128, 4], FP32, name=f"trp_{l}", tag="bc")
    for b in range(4):
        nc.tensor.transpose(trp[:, b:b + 1], r1[:, b * 128:(b + 1) * 128],
                            identf[:])
    rT = spool.tile([128, 4], FP32, name=f"rT_{l}", tag="rT")
    nc.vector.tensor_copy(rT[:], trp[:])

    # ---- qkv GEMMs ----
    qraw = []
    msq_q = ps_acc.tile([16, L], FP32, name=f"msqq_{l}", tag="acc")
    msq_k = ps_acc.tile([16, L], FP32, name=f"msqk_{l}", tag="acc")
    for grp in range(4):
        wt = wpool.tile([128, 4, D], FP16, name=f"qkw_{l}_{grp}", tag="w")
        nc.sync.dma_start(wt[:], d["qk_t"][l, grp * 4:(grp + 1) * 4]
                          .rearrange("g p n -> p g n"))
        for gi in range(4):
            ot = grp * 4 + gi
            ps = ps_g.tile([128, L], FP32, name=f"qk_ps_{l}_{ot}", tag="g")
            for k in range(KT):
                nc.tensor.matmul(ps[:], wt[:, gi, k * 128:(k + 1) * 128],
                                 x16[k][:], start=(k == 0), stop=(k == KT - 1))
            qr = spool.tile([128, L], FP16, name=f"qraw_{l}_{ot}", tag="qraw",
                            bufs=16)
            nc.vector.tensor_copy(qr[:], ps[:])
            qraw.append(qr)
            sq = spool.tile([128, L], FP16, name=f"qsq_{l}_{ot}", tag="sq",
                            bufs=2)
            nc.scalar.activation(sq[:], ps[:], F.Square)
            tt = ot % 8
            msq = msq_q if ot < 8 else msq_k
            nc.tensor.matmul(msq[:], b16[:, tt * 16:(tt + 1) * 16], sq[:],
                             start=(tt == 0), stop=False)
    # eps correction: msq += ep9 * zs  (= eps * ir2)
    nc.tensor.matmul(msq_q[:], ep9[:], zs[:], start=False, stop=True)
    nc.tensor.matmul(msq_k[:], ep9[:], zs[:], start=False, stop=True)

    # v (token-major) with r1 scaling
    vsb = [apool.tile([128, 16 * 65], FP16, name=f"vsb_{l}_{b}", tag=f"vsb{b}")
           for b in range(4)]
    wva = wpool.tile([128, 4, D], FP16, name=f"vwa_{l}", tag="w")
    nc.sync.dma_start(wva[:], d["wv_p"][l, 0:4].rearrange("k p n -> p k n"))
    wvb = wpool.tile([128, 4, D], FP16, name=f"vwb_{l}", tag="w")
    nc.sync.dma_start(wvb[:], d["wv_p"][l, 4:8].rearrange("k p n -> p k n"))
    for b in range(4):
        nc.gpsimd.memset(
            vsb[b][:].rearrange("p (h c) -> p h c", c=65)[:, :, 64:65], 1.0)
        for n in range(2):
            ps = ps_g.tile([128, 512], FP32, name=f"v_ps_{l}_{b}_{n}", tag="g")
            for k in range(KT):
                wv = wva if k < 4 else wvb
                nc.tensor.matmul(ps[:], x16[k][:, b * 128:(b + 1) * 128],
                                 wv[:, k % 4, n * 512:(n + 1) * 512],
                                 start=(k == 0), stop=(k == KT - 1))
            dst = vsb[b][:].rearrange("p (h c) -> p h c", c=65)[:, n * 8:(n + 1) * 8, 0:64]
            nc.vector.tensor_scalar_mul(dst, ps[:], rT[:, b:b + 1])

    # ---- per-head q/k norm multipliers ----
    aq = spool.tile([16, L], FP16, name=f"aq_{l}", tag="a16")
    tq = spool.tile([16, L], FP32, name=f"tq_{l}", tag="a32")
    nc.scalar.activation(tq[:], msq_q[:], F.Ln)
    nc.scalar.activation(aq[:], tq[:], F.Exp, scale=-0.5)
    ak = spool.tile([16, L], FP16, name=f"ak_{l}", tag="a16")
    tk = spool.tile([16, L], FP32, name=f"tk_{l}", tag="a32")
    nc.scalar.activation(tk[:], msq_k[:], F.Ln)
    nc.scalar.activation(ak[:], tk[:], F.Exp, scale=-0.5)

    # ---- rope + norm apply ----
    qf = [None] * 16
    for ot in [x for p in zip(range(8), range(8, 16)) for x in p]:
        t = ot % 8
        alpha = aq if ot < 8 else ak
        ci, si = (0, 1) if ot < 8 else (2, 3)
        bc = ps_bc.tile([128, L], FP32, name=f"rbc_{l}_{ot}", tag="bc")
        nc.tensor.matmul(bc[:], e1s[:, t * 128:(t + 1) * 128],
                         alpha[:], start=True, stop=True)
        sw = ps_bc.tile([128, L], FP32, name=f"rsw_{l}_{ot}", tag="bc")
        nc.tensor.matmul(sw[:], psw[:], qraw[ot][:], start=True, stop=True)
        u1 = spool.tile([128, L], FP16, name=f"u1_{l}_{ot}", tag="u1")
        nc.vector.tensor_mul(u1[:], qraw[ot][:], rtab[:, ci, :])
        u2 = spool.tile([128, L], FP16, name=f"u2_{l}_{ot}", tag="u2")
        nc.vector.tensor_mul(u2[:], sw[:], rtab[:, si, :])
        nc.vector.tensor_add(u1[:], u1[:], u2[:])
        qt = apool.tile([128, L], FP16, name=f"qf_{l}_{ot}", tag=f"qf{ot}")
        nc.vector.tensor_mul(qt[:], u1[:], bc[:])
        qf[ot] = qt

    # ---- attention ----
    rden = spool.tile([97, 4 * L], FP16, name=f"rden_{l}", tag="den", bufs=1)
    dstage = spool.tile([97, 4 * L], FP16, name=f"dstage_{l}", tag="dst", bufs=1)
    oun = [apool.tile([128, L], FP16, name=f"oun_{l}_{t}", tag=f"oun{t}")
           for t in range(8)]
    for ti in range(8):
        ha, hb = 2 * ti, 2 * ti + 1
        oea = ps_acc.tile([65, L], FP32, name=f"oe_{l}_{ha}", tag="acc")
        oeb = ps_acc.tile([65, L], FP32, name=f"oe_{l}_{hb}", tag="acc")
        for kt in range(4):
            q0 = 128 * kt
            sta = ps_g.tile([128, L], FP32, name=f"st_{l}_{ha}_{kt}", tag="g")
            stb = ps_g.tile([128, L], FP32, name=f"st_{l}_{hb}_{kt}", tag="g")
            nc.tensor.matmul(sta[:, q0:], qf[8 + ti][0:64, kt * 128:(kt + 1) * 128],
                             qf[ti][0:64, q0:], start=True, stop=True)
            nc.tensor.matmul(stb[:, q0:], qf[8 + ti][64:128, kt * 128:(kt + 1) * 128],
                             qf[ti][64:128, q0:], start=True, stop=True)
            esta = spool.tile([128, L], FP16, name=f"est_{l}_{ha}_{kt}",
                              tag="est", bufs=4)
            estb = spool.tile([128, L], FP16, name=f"estb_{l}_{hb}_{kt}",
                              tag="estb", bufs=4)
            nc.scalar.activation(esta[:, q0:], sta[:, q0:], F.Exp, scale=0.125)
            nc.scalar.activation(estb[:, q0:], stb[:, q0:], F.Exp, scale=0.125)
            nc.any.memset(esta[64:128, q0:q0 + 64], 0.0)
            nc.any.memset(estb[64:128, q0:q0 + 64], 0.0)
            nc.tensor.matmul(oea[:, q0:], vsb[kt][:, ha * 65:(ha + 1) * 65],
                             esta[:, q0:], start=(kt == 0), stop=(kt == 3))
            nc.tensor.matmul(oeb[:, q0:], vsb[kt][:, hb * 65:(hb + 1) * 65],
                             estb[:, q0:], start=(kt == 0), stop=(kt == 3))
        for h, oe in ((ha, oea), (hb, oeb)):
            dr, dc = 32 * (h // 4), (h % 4) * L
            nc.vector.tensor_scalar_mul(dstage[dr:dr + 1, dc:dc + L],
                                        oe[64:65, :], 1.0 / 16.0)
            nc.vector.tensor_copy(oun[ti][(h % 2) * 64:(h % 2) * 64 + 64, :],
                                  oe[0:64, :])

    # normalize + out projection
    lnd = spool.tile([97, 4 * L], FP32, name=f"lnd_{l}", tag="lnd", bufs=1)
    nc.scalar.activation(lnd[:], dstage[:], F.Ln)
    nc.scalar.activation(rden[:], lnd[:], F.Exp, scale=-1.0)
    of = [spool.tile([128, L], FP16, name=f"of_{l}_{t}", tag=f"of{t}", bufs=1)
          for t in range(8)]
    for t in range(8):
        bc = ps_bc.tile([128, L], FP32, name=f"nbc_{l}_{t}", tag="bc")
        for hh in (2 * t, 2 * t + 1):
            dr, dc = 32 * (hh // 4), (hh % 4) * L
            nc.tensor.matmul(bc[(hh % 2) * 64:(hh % 2) * 64 + 64, :],
                             o16c[dr:dr + 1, 0:64],
                             rden[dr:dr + 1, dc:dc + L],
                             start=True, stop=True,
                             tile_position=(dr, (hh % 2) * 64))
        nc.vector.tensor_mul(of[t][:], oun[t][:], bc[:])
    for grp in range(2):
        wt = wpool.tile([128, 4, D], FP16, name=f"wo_{l}_{grp}", tag="w")
        nc.sync.dma_start(wt[:], d["wo_t"][l, grp * 4:(grp + 1) * 4]
                          .rearrange("g p n -> p g n"))
        for gi in range(4):
            t = grp * 4 + gi
            ps = ps_g.tile([128, L], FP32, name=f"xa_ps_{l}_{t}", tag="g")
            for k in range(KT):
                nc.tensor.matmul(ps[:], wt[:, gi, k * 128:(k + 1) * 128],
                                 of[k][:], start=(k == 0), stop=(k == KT - 1))
            nc.vector.tensor_add(xs[t][:], xs[t][:], ps[:])
            nc.vector.tensor_copy(x16[t][:], xs[t][:])

    # ---- MLP ----
    mean2 = rms_mean(x16, 8, f"l{l}b")
    r2 = spool.tile([1, L], FP16, name=f"r2_{l}", tag="r16")
    nc.scalar.activation(r2[:], mean2[:], F.Abs_reciprocal_sqrt, bias=epsb[0:1])
    h2 = [apool.tile([128, L], FP16, name=f"h2_{l}_{t}", tag=f"h2{t}")
          for t in range(8)]
    bch = ps_bc.tile([128, L], FP32, name=f"bch_{l}", tag="bc")
    nc.tensor.matmul(bch[:], ones1[:], r2[:], start=True, stop=True)
    for t in range(8):
        nc.vector.tensor_mul(h2[t][:], x16[t][:], bch[:])

    pj = []
    for j in range(JT):
        wt = wpool.tile([128, 4, D], FP16, name=f"gu_{l}_{j}", tag="w")
        nc.sync.dma_start(wt[:], d["gu_t"][l, j * 4:(j + 1) * 4]
                          .rearrange("g p n -> p g n"))
        pss = []
        for gi in range(4):
            ps = ps_g.tile([128, L], FP32, name=f"gu_ps_{l}_{j}_{gi}", tag="g")
            for k in range(KT):
                nc.tensor.matmul(ps[:], wt[:, gi, k * 128:(k + 1) * 128],
                                 h2[k][:], start=(k == 0), stop=(k == KT - 1))
            pss.append(ps)
        sg1 = spool.tile([128, L], FP16, name=f"sg1_{l}_{j}", tag="sg1")
        nc.scalar.activation(sg1[:], pss[0][:], F.Silu)
        sg2 = spool.tile([128, L], FP16, name=f"sg2_{l}_{j}", tag="sg2")
        nc.scalar.activation(sg2[:], pss[1][:], F.Silu)
        ta = spool.tile([128, L], FP16, name=f"ta_{l}_{j}", tag="ta")
        nc.vector.tensor_mul(ta[:], sg1[:], pss[2][:])
        tb = spool.tile([128, L], FP16, name=f"tb_{l}_{j}", tag="tb")
        nc.vector.tensor_mul(tb[:], sg2[:], pss[3][:])
        p = spool.tile([128, L], FP16, name=f"p_{l}_{j}", tag=f"p{j}", bufs=1)
        nc.vector.tensor_add(p[:], ta[:], tb[:])
        pj.append(p)

    for grp in range(4):
        wt = wpool.tile([128, 2, IHP], FP16, name=f"dn_{l}_{grp}", tag="w")
        nc.sync.dma_start(wt[:], d["dn_t"][l, grp * 2:(grp + 1) * 2]
                          .rearrange("g p n -> p g n"))
        for gi in range(2):
            t = grp * 2 + gi
            ps = ps_g.tile([128, L], FP32, name=f"dn_ps_{l}_{t}", tag="g")
            for j in range(JT):
                nc.tensor.matmul(ps[:], wt[:, gi, j * 128:(j + 1) * 128],
                                 pj[j][:], start=(j == 0), stop=(j == JT - 1))
            nc.vector.tensor_add(xs[t][:], xs[t][:], ps[:])
            nc.vector.tensor_copy(x16[t][:], xs[t][:])


# ----------------------------------------------------------------------
# entry point
# ----------------------------------------------------------------------

def _get_nc(nl=NL):
    if nl not in _CACHE:
        _CACHE[nl] = _build(nl)
    return _CACHE[nl]


def run(inputs, nl=NL, trace=False):
    inputs = {k: np.asarray(v) for k, v in inputs.items()}
    w = _prep_weights(inputs)
    in_maps = []
    for b in range(N_CORES):
        tok = _patchify(inputs["frames"][b]).astype(np.float32)
        x0T = np.zeros((PDP, L), np.float16)
        x0T[:PD] = tok.T.astype(np.float16)
        m = {"x0T": x0T, "qk_t": w["qk_t"], "wv_p": w["wv_p"],
             "wo_t": w["wo_t"], "gu_t": w["gu_t"], "dn_t": w["dn_t"],
             "pe_t": w["pe_t"], "hd_t": w["hd_t"],
             "e1": w["e1"], "esc": w["esc"],
             "cq": w["cq"], "sq": w["sq"], "ck": w["ck"], "sk": w["sk"],
             "psw": w["psw"], "b16": w["b16"]}
        in_maps.append(m)
    nc = _get_nc(nl)
    res = run_bass_kernel_spmd(nc, in_maps, list(range(N_CORES)), trace=trace)
    outs = []
    for b in range(N_CORES):
        tok = res.results[b]["out"].T  # (L, PD)
        outs.append(_unpatchify(tok))
    return np.stack(outs).astype(np.float32), res


def kernel(**inputs) -> np.ndarray:
    out, _ = run(inputs)
    return out
